# revision 2
# baseline (speedup 1.0000x reference)
"""Trainium2 Bass kernel for nn_DistanceModule.

Computes, for h [4,512,64], W [64,64], b/gamma/beta [64]:
    x = LayerNorm(ReLU(h @ W.T + b))          # [B,N,C]
    D[b,i,j,c] = x[b,i,c] * x[b,j,c]
    out = softmax(D, axis=-1)                 # [B,N,N,C] f32 (256 MB)

Sharding: 2048 (b,i) rows split across 8 cores -> 256 rows/core
(core k: batch b=k//2, i in [256*(k%2), 256*(k%2)+256)). Each core
computes x[b] on-chip, then streams its [256, 512, 64] output slice.

Per-core pipeline (all engines overlapped, per (i-tile, j-block) chunk):
  PE    : selector-matmul broadcasts xT row c across 128 partitions (PSUM)
  ScalarE: exp(bcast_c * x_i[:,c]) fused multiply+exp, per c
  VectorE: segmented reduce_sum over c, reciprocal, normalize multiply
  DMA   : 8 MB contiguous store per chunk

Softmax is computed without max-subtraction: LayerNorm bounds |x| by
sqrt(C-1) ~= 7.94, so logits <= 63 and exp <= 2.4e27 < f32 max.
"""

import numpy as np

import concourse.bacc as bacc
import concourse.bass as bass
import concourse.mybir as mybir
import concourse.tile as tile
from concourse.bass_utils import run_bass_kernel_spmd

B, N, C = 4, 512, 64
NCORES = 8
ROWS = 256          # (b,i) rows per core
JBLK = 256          # j-block width
EPS = 1e-5
F32 = mybir.dt.float32

_CACHE = {}


def _build_program():
    nc = bacc.Bacc(
        "TRN2",
        target_bir_lowering=False,
        debug=False,
        enable_asserts=False,
        num_devices=NCORES,
    )

    hT_d = nc.dram_tensor("hT", [C, N], F32, kind="ExternalInput")
    hTi_d = nc.dram_tensor("hTi", [C, ROWS], F32, kind="ExternalInput")
    WT_d = nc.dram_tensor("WT", [C, C], F32, kind="ExternalInput")
    bgb_d = nc.dram_tensor("bgb", [128, 3 * C], F32, kind="ExternalInput")
    sel_d = nc.dram_tensor("sel", [C, C * 128], F32, kind="ExternalInput")
    id_d = nc.dram_tensor("identity", [128, 128], F32, kind="ExternalInput")
    out_d = nc.dram_tensor("out", [ROWS, N * C], F32, kind="ExternalOutput")

    X = mybir.AxisListType.X
    sub = mybir.AluOpType.subtract
    mult = mybir.AluOpType.mult
    Exp = mybir.ActivationFunctionType.Exp
    Sqrt = mybir.ActivationFunctionType.Sqrt

    with tile.TileContext(nc) as tc:
        with tc.tile_pool(name="const", bufs=1) as constp:
            hT = constp.tile([C, N], F32)
            nc.sync.dma_start(hT[:], hT_d[:])
            hTi = constp.tile([C, ROWS], F32)
            nc.sync.dma_start(hTi[:], hTi_d[:])
            WT = constp.tile([C, C], F32)
            nc.sync.dma_start(WT[:], WT_d[:])
            bgb = constp.tile([128, 3 * C], F32)
            nc.sync.dma_start(bgb[:], bgb_d[:])
            sel = constp.tile([C, C * 128], F32)
            nc.sync.dma_start(sel[:], sel_d[:])
            ident = constp.tile([128, 128], F32)
            nc.sync.dma_start(ident[:], id_d[:])

            xT = constp.tile([C, N], F32)          # x[b].T  (c on partitions)
            xi = constp.tile([128, 2, C], F32)     # this core's two i-tiles
            eps_t = constp.tile([128, 1], F32)
            nc.vector.memset(eps_t[:], EPS)

            # ---- x = LayerNorm(ReLU(h @ W.T + b)) --------------------------
            with (
                tc.tile_pool(name="xprep", bufs=2) as xprep,
                tc.tile_pool(name="psum_prep", bufs=2, space=bass.MemorySpace.PSUM) as psp,
                tc.tile_pool(name="psum_tp", bufs=2, space=bass.MemorySpace.PSUM) as ptp,
            ):
                for t in range(6):
                    if t < 4:
                        lhsT = hT[:, t * 128:(t + 1) * 128]
                    else:
                        lhsT = hTi[:, (t - 4) * 128:(t - 3) * 128]
                    xp = psp.tile([128, C], F32, tag="xp")
                    nc.tensor.matmul(xp[:], lhsT, WT[:])
                    xs = xprep.tile([128, C], F32, tag="xs")
                    nc.vector.tensor_add(xs[:], xp[:], bgb[:, 0:C])       # + b
                    nc.vector.tensor_scalar_max(xs[:], xs[:], 0.0)        # ReLU
                    stats = xprep.tile([128, 6], F32, tag="stats")
                    nc.vector.bn_stats(stats[:], xs[:])
                    mv = xprep.tile([128, 2], F32, tag="mv")
                    nc.vector.bn_aggr(mv[:], stats[:])
                    std = xprep.tile([128, 1], F32, tag="std")
                    nc.scalar.activation(std[:], mv[:, 1:2], Sqrt, bias=eps_t[:, 0:1])
                    rstd = xprep.tile([128, 1], F32, tag="rstd")
                    nc.vector.reciprocal(rstd[:], std[:])
                    xn = xprep.tile([128, C], F32, tag="xn")
                    nc.vector.tensor_scalar(
                        xn[:], xs[:], mv[:, 0:1], rstd[:, 0:1], op0=sub, op1=mult
                    )
                    nc.vector.tensor_mul(xn[:], xn[:], bgb[:, C:2 * C])   # * gamma
                    nc.vector.tensor_add(xn[:], xn[:], bgb[:, 2 * C:3 * C])  # + beta
                    if t < 4:
                        tp = ptp.tile([C, 128], F32, tag="tp")
                        nc.tensor.transpose(tp[:], xn[:], ident[:])
                        nc.vector.tensor_copy(xT[:, t * 128:(t + 1) * 128], tp[:])
                    else:
                        nc.vector.tensor_copy(xi[:, t - 4, :], xn[:])

            # ---- main: exp(x_i * x_j), softmax over c, store ---------------
            with (
                tc.tile_pool(name="main", bufs=2) as mainp,
                tc.tile_pool(name="small", bufs=3) as smallp,
                tc.tile_pool(name="psum_bc", bufs=4, space=bass.MemorySpace.PSUM) as pbc,
            ):
                for it in range(2):
                    for jb in range(N // JBLK):
                        expt = mainp.tile([128, JBLK, C], F32, tag="exp")
                        for c in range(C):
                            bc = pbc.tile([128, JBLK], F32, tag="bc")
                            nc.tensor.matmul(
                                bc[:],
                                sel[:, c * 128:(c + 1) * 128],
                                xT[:, jb * JBLK:(jb + 1) * JBLK],
                            )
                            nc.scalar.activation(
                                expt[:, :, c], bc[:], Exp, scale=xi[:, it, c:c + 1]
                            )
                        sums = smallp.tile([128, JBLK], F32, tag="sums")
                        nc.vector.reduce_sum(sums[:], expt[:], axis=X)
                        recip = smallp.tile([128, JBLK], F32, tag="recip")
                        nc.vector.reciprocal(recip[:], sums[:])
                        nc.vector.tensor_mul(
                            expt[:],
                            expt[:],
                            recip[:, :, None].broadcast_to((128, JBLK, C)),
                        )
                        nc.sync.dma_start(
                            out_d[it * 128:(it + 1) * 128,
                                  jb * JBLK * C:(jb + 1) * JBLK * C],
                            expt[:].rearrange("p j c -> p (j c)"),
                        )
    nc.compile()
    return nc


def _in_maps(h, W, b, gamma, beta):
    h = np.asarray(h, dtype=np.float32)
    W = np.asarray(W, dtype=np.float32)
    b = np.asarray(b, dtype=np.float32)
    gamma = np.asarray(gamma, dtype=np.float32)
    beta = np.asarray(beta, dtype=np.float32)

    WT = np.ascontiguousarray(W.T)
    bgb = np.ascontiguousarray(
        np.broadcast_to(np.concatenate([b, gamma, beta])[None, :], (128, 3 * C))
    )
    sel = np.zeros((C, C * 128), dtype=np.float32)
    for c in range(C):
        sel[c, c * 128:(c + 1) * 128] = 1.0
    ident = np.eye(128, dtype=np.float32)

    in_maps = []
    for k in range(NCORES):
        bb, half = divmod(k, 2)
        i0 = half * ROWS
        in_maps.append({
            "hT": np.ascontiguousarray(h[bb].T),
            "hTi": np.ascontiguousarray(h[bb, i0:i0 + ROWS].T),
            "WT": WT,
            "bgb": bgb,
            "sel": sel,
            "identity": ident,
        })
    return in_maps


def run(h, W, b, gamma, beta, trace=False, **trace_kwargs):
    if "nc" not in _CACHE:
        _CACHE["nc"] = _build_program()
    nc = _CACHE["nc"]
    res = run_bass_kernel_spmd(
        nc,
        _in_maps(h, W, b, gamma, beta),
        core_ids=list(range(NCORES)),
        trace=trace,
        **trace_kwargs,
    )
    out = np.zeros((B, N, N, C), dtype=np.float32)
    for k in range(NCORES):
        bb, half = divmod(k, 2)
        i0 = half * ROWS
        out[bb, i0:i0 + ROWS] = res.results[k]["out"].reshape(ROWS, N, C)
    return out, res


def kernel(h, W, b, gamma, beta):
    out, _ = run(h, W, b, gamma, beta)
    return out


# revision 8
# speedup vs baseline: 1.2152x; 1.2152x over previous
"""Trainium2 Bass kernel for nn_DistanceModule.

Computes, for h [4,512,64], W [64,64], b/gamma/beta [64]:
    x = LayerNorm(ReLU(h @ W.T + b))          # [B,N,C]
    D[b,i,j,c] = x[b,i,c] * x[b,j,c]
    out = softmax(D, axis=-1)                 # [B,N,N,C] f32 (256 MB)

Sharding: 2048 (b,i) rows split across 8 cores -> 256 rows/core
(core k: batch b=k//2, i in [256*(k%2), 256*(k%2)+256)). Each core
computes x[b] on-chip, then streams its [256, 512, 64] output slice.

Per-core pipeline (all engines overlapped, per (i-tile, j-block) chunk):
  PE    : selector-matmul broadcasts xT row c across 128 partitions (PSUM)
  ScalarE: exp(bcast_c * x_i[:,c]) fused multiply+exp, per c
  VectorE: segmented reduce_sum over c, reciprocal, normalize multiply
  DMA   : 8 MB contiguous store per chunk

Softmax is computed without max-subtraction: LayerNorm bounds |x| by
sqrt(C-1) ~= 7.94, so logits <= 63 and exp <= 2.4e27 < f32 max.
"""

import numpy as np

import concourse.bacc as bacc
import concourse.bass as bass
import concourse.mybir as mybir
import concourse.tile as tile
from concourse.bass_utils import run_bass_kernel_spmd

B, N, C = 4, 512, 64
NCORES = 8
ROWS = 256          # (b,i) rows per core
JBLK = 256          # j-block width
EPS = 1e-5
F32 = mybir.dt.float32
BF16 = mybir.dt.bfloat16

_CACHE = {}


def _build_program():
    nc = bacc.Bacc(
        "TRN2",
        target_bir_lowering=False,
        debug=False,
        enable_asserts=False,
        num_devices=NCORES,
    )

    hT_d = nc.dram_tensor("hT", [C, N], F32, kind="ExternalInput")
    hTi_d = nc.dram_tensor("hTi", [C, ROWS], F32, kind="ExternalInput")
    WT_d = nc.dram_tensor("WT", [C, C], F32, kind="ExternalInput")
    bgb_d = nc.dram_tensor("bgb", [128, 3 * C], F32, kind="ExternalInput")
    sel_d = nc.dram_tensor("sel", [C, C * 128], BF16, kind="ExternalInput")
    id_d = nc.dram_tensor("identity", [128, 128], F32, kind="ExternalInput")
    out_d = nc.dram_tensor("out", [ROWS, N * C], F32, kind="ExternalOutput")

    X = mybir.AxisListType.X
    sub = mybir.AluOpType.subtract
    mult = mybir.AluOpType.mult
    Exp = mybir.ActivationFunctionType.Exp
    Sqrt = mybir.ActivationFunctionType.Sqrt

    with tile.TileContext(nc) as tc:
        with tc.tile_pool(name="const", bufs=1) as constp:
            hT = constp.tile([C, N], F32)
            nc.sync.dma_start(hT[:], hT_d[:])
            hTi = constp.tile([C, ROWS], F32)
            nc.sync.dma_start(hTi[:], hTi_d[:])
            WT = constp.tile([C, C], F32)
            nc.sync.dma_start(WT[:], WT_d[:])
            bgb = constp.tile([128, 3 * C], F32)
            nc.sync.dma_start(bgb[:], bgb_d[:])
            sel = constp.tile([C, C * 128], BF16)
            nc.sync.dma_start(sel[:], sel_d[:])
            ident = constp.tile([128, 128], F32)
            nc.sync.dma_start(ident[:], id_d[:])

            xT = constp.tile([C, N], F32)          # x[b].T  (c on partitions)
            xi = constp.tile([128, 2, C], F32)     # this core's two i-tiles
            eps_t = constp.tile([128, 1], F32)
            nc.vector.memset(eps_t[:], EPS)

            # ---- x = LayerNorm(ReLU(h @ W.T + b)) --------------------------
            with (
                tc.tile_pool(name="xprep", bufs=2) as xprep,
                tc.tile_pool(name="psum_prep", bufs=2, space=bass.MemorySpace.PSUM) as psp,
                tc.tile_pool(name="psum_tp", bufs=2, space=bass.MemorySpace.PSUM) as ptp,
            ):
                for t in range(6):
                    if t < 4:
                        lhsT = hT[:, t * 128:(t + 1) * 128]
                    else:
                        lhsT = hTi[:, (t - 4) * 128:(t - 3) * 128]
                    xp = psp.tile([128, C], F32, tag="xp")
                    nc.tensor.matmul(xp[:], lhsT, WT[:])
                    xs = xprep.tile([128, C], F32, tag="xs")
                    nc.vector.tensor_add(xs[:], xp[:], bgb[:, 0:C])       # + b
                    nc.vector.tensor_scalar_max(xs[:], xs[:], 0.0)        # ReLU
                    stats = xprep.tile([128, 6], F32, tag="stats")
                    nc.vector.bn_stats(stats[:], xs[:])
                    mv = xprep.tile([128, 2], F32, tag="mv")
                    nc.vector.bn_aggr(mv[:], stats[:])
                    std = xprep.tile([128, 1], F32, tag="std")
                    nc.scalar.activation(std[:], mv[:, 1:2], Sqrt, bias=eps_t[:, 0:1])
                    rstd = xprep.tile([128, 1], F32, tag="rstd")
                    nc.vector.reciprocal(rstd[:], std[:])
                    xn = xprep.tile([128, C], F32, tag="xn")
                    nc.vector.tensor_scalar(
                        xn[:], xs[:], mv[:, 0:1], rstd[:, 0:1], op0=sub, op1=mult
                    )
                    nc.vector.tensor_mul(xn[:], xn[:], bgb[:, C:2 * C])   # * gamma
                    nc.vector.tensor_add(xn[:], xn[:], bgb[:, 2 * C:3 * C])  # + beta
                    if t < 4:
                        tp = ptp.tile([C, 128], F32, tag="tp")
                        nc.tensor.transpose(tp[:], xn[:], ident[:])
                        nc.vector.tensor_copy(xT[:, t * 128:(t + 1) * 128], tp[:])
                    else:
                        nc.vector.tensor_copy(xi[:, t - 4, :], xn[:])

            # hi/lo bf16 split of xT: x = hi + lo exactly to ~2^-17, so the
            # bf16 matmul pair (PSUM accumulates in fp32) reproduces the f32
            # broadcast at ~4x the fp32 matmul speed.
            xT_hi = constp.tile([C, N], BF16)
            nc.vector.tensor_copy(xT_hi[:], xT[:])
            hi32 = constp.tile([C, N], F32)
            nc.vector.tensor_copy(hi32[:], xT_hi[:])
            xT_lo = constp.tile([C, N], BF16)
            nc.vector.tensor_sub(xT_lo[:], xT[:], hi32[:])

            # ---- main: exp(x_i * x_j), softmax over c, store ---------------
            with (
                tc.tile_pool(name="main", bufs=2) as mainp,
                tc.tile_pool(name="small", bufs=3) as smallp,
                tc.tile_pool(name="psum_bc", bufs=4, space=bass.MemorySpace.PSUM) as pbc,
            ):
                for it in range(2):
                    for jb in range(N // JBLK):
                        expt = mainp.tile([128, JBLK, C], F32, tag="exp")
                        for c in range(C):
                            bc = pbc.tile([128, JBLK], F32, tag="bc")
                            nc.tensor.matmul(
                                bc[:],
                                sel[:, c * 128:(c + 1) * 128],
                                xT_hi[:, jb * JBLK:(jb + 1) * JBLK],
                                start=True, stop=False,
                            )
                            nc.tensor.matmul(
                                bc[:],
                                sel[:, c * 128:(c + 1) * 128],
                                xT_lo[:, jb * JBLK:(jb + 1) * JBLK],
                                start=False, stop=True,
                            )
                            nc.scalar.activation(
                                expt[:, :, c], bc[:], Exp, scale=xi[:, it, c:c + 1]
                            )
                        sums = smallp.tile([128, JBLK], F32, tag="sums")
                        nc.vector.reduce_sum(sums[:], expt[:], axis=X)
                        recip = smallp.tile([128, JBLK], F32, tag="recip")
                        nc.vector.reciprocal(recip[:], sums[:])
                        nc.vector.tensor_mul(
                            expt[:],
                            expt[:],
                            recip[:, :, None].broadcast_to((128, JBLK, C)),
                        )
                        nc.sync.dma_start(
                            out_d[it * 128:(it + 1) * 128,
                                  jb * JBLK * C:(jb + 1) * JBLK * C],
                            expt[:].rearrange("p j c -> p (j c)"),
                        )
    nc.compile()
    return nc


def _in_maps(h, W, b, gamma, beta):
    h = np.asarray(h, dtype=np.float32)
    W = np.asarray(W, dtype=np.float32)
    b = np.asarray(b, dtype=np.float32)
    gamma = np.asarray(gamma, dtype=np.float32)
    beta = np.asarray(beta, dtype=np.float32)

    WT = np.ascontiguousarray(W.T)
    bgb = np.ascontiguousarray(
        np.broadcast_to(np.concatenate([b, gamma, beta])[None, :], (128, 3 * C))
    )
    import ml_dtypes
    sel = np.zeros((C, C * 128), dtype=ml_dtypes.bfloat16)
    for c in range(C):
        sel[c, c * 128:(c + 1) * 128] = 1.0
    ident = np.eye(128, dtype=np.float32)

    in_maps = []
    for k in range(NCORES):
        bb, half = divmod(k, 2)
        i0 = half * ROWS
        in_maps.append({
            "hT": np.ascontiguousarray(h[bb].T),
            "hTi": np.ascontiguousarray(h[bb, i0:i0 + ROWS].T),
            "WT": WT,
            "bgb": bgb,
            "sel": sel,
            "identity": ident,
        })
    return in_maps


def run(h, W, b, gamma, beta, trace=False, **trace_kwargs):
    if "nc" not in _CACHE:
        _CACHE["nc"] = _build_program()
    nc = _CACHE["nc"]
    res = run_bass_kernel_spmd(
        nc,
        _in_maps(h, W, b, gamma, beta),
        core_ids=list(range(NCORES)),
        trace=trace,
        **trace_kwargs,
    )
    out = np.zeros((B, N, N, C), dtype=np.float32)
    for k in range(NCORES):
        bb, half = divmod(k, 2)
        i0 = half * ROWS
        out[bb, i0:i0 + ROWS] = res.results[k]["out"].reshape(ROWS, N, C)
    return out, res


def kernel(h, W, b, gamma, beta):
    out, _ = run(h, W, b, gamma, beta)
    return out
